# revision 20
# baseline (speedup 1.0000x reference)
"""Trainium2 Bass kernel for nn_CortexReasoner (moe_routing).

Sharding across 8 NeuronCores:
  - Attention: T=4096 tokens sharded 512/core; K,V all-gathered (one fused AG).
  - Scores computed in [u, t] layout (u on partitions) so softmax'd probs feed
    the PV matmul directly with no transposes; per-row max via PE-transpose of
    the running column-max + exp applied straight out of PSUM with a
    (-max)-broadcast PSUM init matmul.
  - Gate/top-k/router aggregation are pure functions of the inputs -> host.
    RWKV step computed ONLY for active regions (<=10 of 32), e-dim sharded
    128/core; partial h AllReduced.
  - Output head: Wf e-sharded, W_out vocab-sharded 6400/core.
Matmuls run as float32r (hw-verified ~1.5e-4 relmax, full rate at N>=256).
"""
import sys
sys.path.insert(0, "/opt/trn_rl_repo")

import numpy as np
import concourse.bass as bass
import concourse.bacc as bacc
import concourse.tile as tile
import concourse.mybir as mybir
from concourse.bass_utils import run_bass_kernel_spmd

R, D, V, T, NBR = 32, 1024, 50257, 4096, 8
KF, NH, KACT, KINNER = 8, 4, 8, 8
IO_S, IO_M = 0, 1
HD = D // NH            # 256
NCORES = 8
TQ = T // NCORES        # 512 q rows per core
KT = D // 128           # 8 contraction chunks
NU = T // 128           # 32 u-tiles
VPAD = 51200
VC = VPAD // NCORES     # 6400
NVT = VC // 128         # 50
EC = D // NCORES        # 128 (e-chunk per core)
KVN = D * TQ

f32 = mybir.dt.float32
f32r = mybir.dt.float32r
AF = mybir.ActivationFunctionType
ALU = mybir.AluOpType
AX = mybir.AxisListType

_prog_cache = {}


def _build(A: int):
    nc = bacc.Bacc("TRN2", target_bir_lowering=False, debug=False,
                   num_devices=NCORES)
    t = {}

    def din(name, shape, dt=f32r):
        t[name] = nc.dram_tensor(name, shape, dt, kind="ExternalInput")

    din("xT", [D, TQ]); din("fsm", [2, TQ]); din("onz", [2, 128])
    din("Wq", [D, D]); din("Wk", [D, D]); din("Wv_at", [D, D]); din("Wo_at", [D, D])
    din("ones", [128, 128]); din("ident", [128, 128], f32)
    din("gammaR", [1, D], f32); din("MioR", [2, D], f32)
    din("XinNT", [128, KT, A], f32)
    din("WrC", [A, KT, 128, 128], f32); din("WvC", [A, KT, 128, 128], f32)
    din("WoC", [A, KT, 128, 128], f32); din("bstepC", [128, A], f32)
    din("wpiT", [D, KF], f32); din("WfC", [KF, KT, 128, 128], f32)
    din("WoutC", [D, VC], f32)

    t["logits"] = nc.dram_tensor("logits", [VC], f32, kind="ExternalOutput")
    t["sensor"] = nc.dram_tensor("sensor", [D], f32, kind="ExternalOutput")
    t["hsum"] = nc.dram_tensor("hsum", [A, D], f32, kind="ExternalOutput")
    t["dbg_pi"] = nc.dram_tensor("dbg_pi", [KF], f32, kind="ExternalOutput")
    t["dbg_u"] = nc.dram_tensor("dbg_u", [EC], f32, kind="ExternalOutput")
    t["dbg_motor"] = nc.dram_tensor("dbg_motor", [D], f32, kind="ExternalOutput")
    t["dbg_F"] = nc.dram_tensor("dbg_F", [128, KF], f32, kind="ExternalOutput")

    t["kb"] = nc.dram_tensor("k_bounce", [KVN], f32r)
    t["kag"] = nc.dram_tensor("k_ag", [NCORES, KVN], f32r, addr_space="Shared")
    t["vb"] = nc.dram_tensor("v_bounce", [KVN], f32r)
    t["vag"] = nc.dram_tensor("v_ag", [NCORES, KVN], f32r, addr_space="Shared")
    t["senb"] = nc.dram_tensor("sens_bounce", [D], f32)
    t["senar"] = nc.dram_tensor("sens_ar", [D], f32, addr_space="Shared")
    t["hpb"] = nc.dram_tensor("hp_bounce", [A, D], f32)
    t["hpar"] = nc.dram_tensor("hp_ar", [A, D], f32, addr_space="Shared")
    t["ub"] = nc.dram_tensor("u_bounce", [EC], f32)
    t["uag"] = nc.dram_tensor("u_ag", [D], f32, addr_space="Shared")
    t["xiob"] = nc.dram_tensor("xio_bounce", [2, D], f32)
    t["pib"] = nc.dram_tensor("pi_bounce", [KF], f32)
    t["RG"] = [list(range(NCORES))]

    with tile.TileContext(nc) as tc:
        _emit(nc, tc, A, t)
    nc.compile()
    return nc


def _emit(nc, tc, A, t):
    from contextlib import ExitStack
    ctx = ExitStack()
    with ctx:
        sb = ctx.enter_context(tc.tile_pool(name="sb", bufs=1))
        wpool = ctx.enter_context(tc.tile_pool(name="w", bufs=4))
        big_ps = ctx.enter_context(tc.tile_pool(name="bigps", bufs=2, space="PSUM"))
        rwk_ps = ctx.enter_context(tc.tile_pool(name="rwkps", bufs=2, space="PSUM"))
        ppool = ctx.enter_context(tc.tile_pool(name="p", bufs=2))
        pp3 = ctx.enter_context(tc.tile_pool(name="pp3", bufs=3))
        rowp = ctx.enter_context(tc.tile_pool(name="rowp", bufs=1))
        rwk_w = ctx.enter_context(tc.tile_pool(name="rwkw", bufs=4))

        # ---- constants / small loads ----
        ones = sb.tile([128, 128], f32r)
        nc.sync.dma_start(ones[:], t["ones"][:])
        onz = sb.tile([2, 128], f32r)
        nc.sync.dma_start(onz[:], t["onz"][:])
        ident = sb.tile([128, 128], f32)
        nc.sync.dma_start(ident[:], t["ident"][:])
        fsm = sb.tile([2, TQ], f32r)
        nc.sync.dma_start(fsm[:], t["fsm"][:])
        xT = sb.tile([128, KT, TQ], f32r)
        nc.sync.dma_start(xT[:], t["xT"][:].rearrange("(kt p) t -> p kt t", p=128))
        gamma = sb.tile([1, D], f32)
        nc.sync.dma_start(gamma[:], t["gammaR"][:])
        Mio = sb.tile([1, 2 * D], f32)
        nc.sync.dma_start(Mio[:], t["MioR"][:].rearrange("r d -> (r d)").unsqueeze(0))
        XinN = sb.tile([128, KT, A], f32)
        nc.sync.dma_start(XinN[:], t["XinNT"][:])
        bstep = sb.tile([128, A], f32)
        nc.sync.dma_start(bstep[:], t["bstepC"][:])
        wpiT = sb.tile([128, KT, KF], f32)
        nc.sync.dma_start(wpiT[:], t["wpiT"][:].rearrange("(kt p) k -> p kt k", p=128))

        hT_sb = sb.tile([128, A, KT], f32)

        def rwkv_region(a, xin_tile, xin_col):
            rhs = xin_tile[:, :, xin_col]
            rg_ps = rwk_ps.tile([128, NVT], f32, tag="rwk")
            for kt in range(KT):
                wr = rwk_w.tile([128, 128], f32, tag="wr")
                nc.sync.dma_start(wr[:], t["WrC"][a, kt])
                nc.tensor.matmul(rg_ps[:, 0:1], wr[:], rhs[:, kt:kt+1],
                                 start=(kt == 0), stop=(kt == KT - 1))
            vv_ps = rwk_ps.tile([128, NVT], f32, tag="rwk")
            for kt in range(KT):
                wv = rwk_w.tile([128, 128], f32, tag="wv")
                nc.sync.dma_start(wv[:], t["WvC"][a, kt])
                nc.tensor.matmul(vv_ps[:, 0:1], wv[:], rhs[:, kt:kt+1],
                                 start=(kt == 0), stop=(kt == KT - 1))
            rg = ppool.tile([128, 1], f32, tag="rg")
            nc.scalar.activation(rg[:], rg_ps[:, 0:1], AF.Sigmoid,
                                 bias=bstep[:, a:a+1])
            rgvv = ppool.tile([128, 1], f32, tag="rgvv")
            nc.vector.tensor_tensor(rgvv[:], rg[:], vv_ps[:, 0:1], op=ALU.mult)
            h_ps = rwk_ps.tile([128, NVT], f32, tag="rwk")
            for mt in range(KT):
                wo = rwk_w.tile([128, 128], f32, tag="wo")
                nc.sync.dma_start(wo[:], t["WoC"][a, mt])
                nc.tensor.matmul(h_ps[:, mt:mt+1], wo[:], rgvv[:],
                                 start=True, stop=True)
            nc.vector.tensor_copy(hT_sb[:, a, :], h_ps[:, 0:KT])
            nc.sync.dma_start(
                t["hpb"][a:a+1, :].rearrange("one (kt p) -> (one p) kt", p=128),
                hT_sb[:, a, :])

        for a in range(2, A):
            rwkv_region(a, XinN, a)

        # =========== attention projections ===========
        qT = sb.tile([128, KT, TQ], f32r)
        kvctx = ExitStack()
        kvpool = kvctx.enter_context(tc.tile_pool(name="kv", bufs=1))
        if True:
            kT_t = kvpool.tile([128, KT, TQ], f32r, tag="kh", name="kT_t")
            for (W_n, dst) in (("Wk", kT_t), ("Wq", qT)):
                for m in range(KT):
                    ps = big_ps.tile([128, TQ], f32, tag="big")
                    for kt in range(KT):
                        wt = wpool.tile([128, 128], f32r, tag="wt")
                        nc.sync.dma_start(
                            wt[:], t[W_n][kt*128:(kt+1)*128, m*128:(m+1)*128])
                        nc.tensor.matmul(ps[:], wt[:], xT[:, kt, :],
                                         start=(kt == 0), stop=(kt == KT - 1))
                    nc.vector.tensor_copy(dst[:, m, :], ps[:])
                if W_n == "Wk":
                    nc.sync.dma_start(
                        t["kb"][:].rearrange("(m p tt) -> p m tt", m=KT, p=128),
                        kT_t[:])
                    nc.gpsimd.collective_compute(
                        "AllGather", ALU.bypass, replica_groups=t["RG"],
                        ins=[t["kb"][:].opt()], outs=[t["kag"][:].opt()])
            v_t = kvpool.tile([128, 4, D], f32r, tag="vh", name="v_t")
            for tm in range(4):
                for nh in range(2):
                    ps = big_ps.tile([128, 512], f32, tag="big")
                    for kt in range(KT):
                        wt = wpool.tile([128, 512], f32r, tag="wtv")
                        nc.sync.dma_start(wt[:], t["Wv_at"][kt*128:(kt+1)*128,
                                                            nh*512:(nh+1)*512])
                        nc.tensor.matmul(ps[:], xT[:, kt, tm*128:(tm+1)*128], wt[:],
                                         start=(kt == 0), stop=(kt == KT - 1))
                    nc.vector.tensor_copy(v_t[:, tm, nh*512:(nh+1)*512], ps[:])
            nc.sync.dma_start(
                t["vb"][:].rearrange("(tm p d) -> p tm d", tm=4, p=128),
                v_t[:])
        nc.gpsimd.collective_compute(
            "AllGather", ALU.bypass, replica_groups=t["RG"],
            ins=[t["vb"][:].opt()], outs=[t["vag"][:].opt()])

        # =========== attention heads ===========
        oT = sb.tile([128, KT, TQ], f32r)
        with tc.tile_pool(name="attacc", bufs=1, space="PSUM") as att_ps:
            for h in range(NH):
                Kh = kvpool.tile([128, 2, NCORES, TQ], f32r, tag="kh")
                Vh = kvpool.tile([128, NU, HD], f32r, tag="vh")
                for cb in range(NCORES):
                    kblk = t["kag"][cb, :].rearrange(
                        "(d tt) -> d tt", d=D)[h*HD:(h+1)*HD, :]
                    nc.sync.dma_start(
                        Kh[:, :, cb, :],
                        kblk.rearrange("(dd p) tt -> p dd tt", p=128))
                    vblk = t["vag"][cb, :].rearrange(
                        "(u d) -> u d", d=D)[:, h*HD:(h+1)*HD]
                    nc.sync.dma_start(
                        Vh[:, cb*4:(cb+1)*4, :],
                        vblk.rearrange("(tl p) d -> p tl d", p=128))
                qh0 = qT[:, 2*h, :]
                qh1 = qT[:, 2*h+1, :]

                def kslice(dd, ut):
                    return Kh[:, dd, ut // 4, (ut % 4)*128:(ut % 4 + 1)*128]

                # pass 1: scores + running col-max
                smax = ppool.tile([128, TQ], f32, tag="smax", bufs=1)
                for ut in range(NU):
                    s_ps = big_ps.tile([128, TQ], f32, tag="big")
                    nc.tensor.matmul(s_ps[:], kslice(0, ut), qh0, start=True, stop=False)
                    nc.tensor.matmul(s_ps[:], kslice(1, ut), qh1, start=False, stop=True)
                    if ut == 0:
                        nc.vector.tensor_copy(smax[:], s_ps[:])
                    else:
                        nc.vector.tensor_tensor(smax[:], smax[:], s_ps[:], op=ALU.max)
                mrow = ppool.tile([2, TQ], f32, tag="mrow", bufs=1)
                for tck in range(4):
                    tr = big_ps.tile([128, 128], f32, tag="big")
                    nc.tensor.transpose(tr[:], smax[:, tck*128:(tck+1)*128], ident[:])
                    mcol = ppool.tile([128, 1], f32, tag="mcol")
                    nc.vector.tensor_reduce(mcol[:], tr[:], axis=AX.X, op=ALU.max)
                    nc.sync.dma_start(mrow[0:1, tck*128:(tck+1)*128], mcol[:])
                    nc.sync.dma_start(mrow[1:2, tck*128:(tck+1)*128], mcol[:])
                mneg = ppool.tile([2, TQ], f32r, tag="mneg", bufs=1)
                nc.vector.tensor_scalar(out=mneg[:], in0=mrow[:], scalar1=-1.0,
                                        scalar2=None, op0=ALU.mult)
                # pass 2
                l_ps = att_ps.tile([2, TQ], f32, tag="l")
                o_ps0 = att_ps.tile([128, TQ], f32, tag="o0")
                o_ps1 = att_ps.tile([128, TQ], f32, tag="o1")
                for ut in range(NU):
                    s_ps = big_ps.tile([128, TQ], f32, tag="big")
                    nc.tensor.matmul(s_ps[:], onz[:], mneg[:], start=True, stop=False)
                    nc.tensor.matmul(s_ps[:], kslice(0, ut), qh0, start=False, stop=False)
                    nc.tensor.matmul(s_ps[:], kslice(1, ut), qh1, start=False, stop=True)
                    p = pp3.tile([128, TQ], f32r, tag="p")
                    nc.scalar.activation(p[:], s_ps[:], AF.Exp)
                    nc.tensor.matmul(l_ps[:], ones[:, 0:2], p[:],
                                     start=(ut == 0), stop=(ut == NU - 1))
                    nc.tensor.matmul(o_ps0[:], Vh[:, ut, 0:128], p[:],
                                     start=(ut == 0), stop=(ut == NU - 1))
                    nc.tensor.matmul(o_ps1[:], Vh[:, ut, 128:256], p[:],
                                     start=(ut == 0), stop=(ut == NU - 1))
                linv = ppool.tile([2, TQ], f32r, tag="linv")
                with nc.allow_low_precision(reason="f32r rounding feeds matmul"):
                    nc.vector.reciprocal(linv[:], l_ps[:])
                lbc_ps = big_ps.tile([128, TQ], f32, tag="big")
                nc.tensor.matmul(lbc_ps[:], onz[:], linv[:], start=True, stop=True)
                lbc = ppool.tile([128, TQ], f32, tag="lbcs", bufs=1)
                nc.vector.tensor_copy(lbc[:], lbc_ps[:])
                nc.vector.tensor_tensor(oT[:, 2*h, :], o_ps0[:], lbc[:], op=ALU.mult)
                nc.vector.tensor_tensor(oT[:, 2*h+1, :], o_ps1[:], lbc[:], op=ALU.mult)

        kvctx.close()
        # ---- o-proj + residual + sensor partial ----
        fsm_ps = big_ps.tile([128, TQ], f32, tag="big")
        nc.tensor.matmul(fsm_ps[:], onz[:], fsm[:], start=True, stop=True)
        fsm_bc = sb.tile([128, TQ], f32)
        nc.vector.tensor_copy(fsm_bc[:], fsm_ps[:])
        sens = sb.tile([128, KT], f32)
        for m in range(KT):
            ps = big_ps.tile([128, TQ], f32, tag="big")
            for kt in range(KT):
                wt = wpool.tile([128, 128], f32r, tag="wt")
                nc.sync.dma_start(wt[:], t["Wo_at"][kt*128:(kt+1)*128, m*128:(m+1)*128])
                nc.tensor.matmul(ps[:], wt[:], oT[:, kt, :],
                                 start=(kt == 0), stop=(kt == KT - 1))
            mix = ppool.tile([128, TQ], f32, tag="mix")
            nc.vector.tensor_tensor(mix[:], ps[:], xT[:, m, :], op=ALU.add)
            mixf = ppool.tile([128, TQ], f32, tag="mixf")
            nc.vector.tensor_tensor(mixf[:], mix[:], fsm_bc[:], op=ALU.mult)
            nc.vector.tensor_reduce(sens[:, m:m+1], mixf[:], axis=AX.X, op=ALU.add)
        nc.sync.dma_start(
            t["senb"][:].rearrange("(m p) -> p m", p=128),
            sens[:])
        nc.gpsimd.collective_compute(
            "AllReduce", ALU.add, replica_groups=t["RG"],
            ins=[t["senb"][:].opt()], outs=[t["senar"][:].opt()])
        nc.sync.dma_start(t["sensor"][:], t["senar"][:])

        # ---- Xin rows 0,1 ----
        sens_row = sb.tile([1, D], f32)
        nc.sync.dma_start(sens_row[:], t["senar"][:].unsqueeze(0))
        eps_t = sb.tile([1, 1], f32)
        nc.vector.memset(eps_t[:], 1e-6)
        XinIO = sb.tile([128, KT, 2], f32)
        for r in range(2):
            y = rowp.tile([1, D], f32, tag="y")
            nc.vector.tensor_tensor(y[:], Mio[0:1, r*D:(r+1)*D], sens_row[:], op=ALU.add)
            ysq = rowp.tile([1, D], f32, tag="ysq")
            ssq = rowp.tile([1, 1], f32, tag="ssq")
            nc.scalar.activation(ysq[:], y[:], AF.Square, accum_out=ssq[:])
            sd = rowp.tile([1, 1], f32, tag="sd")
            nc.scalar.activation(sd[:], ssq[:], AF.Sqrt, bias=eps_t[:], scale=1.0 / D)
            rinv = rowp.tile([1, 1], f32, tag="rinv")
            nc.vector.reciprocal(rinv[:], sd[:])
            yn = rowp.tile([1, D], f32, tag="yn")
            nc.vector.tensor_scalar(out=yn[:], in0=y[:], scalar1=rinv[:],
                                    scalar2=None, op0=ALU.mult)
            yg = rowp.tile([1, D], f32, tag="yg")
            nc.vector.tensor_tensor(yg[:], yn[:], gamma[:], op=ALU.mult)
            nc.sync.dma_start(t["xiob"][r:r+1, :], yg[:])
            nc.sync.dma_start(
                XinIO[:, :, r],
                t["xiob"][r:r+1, :].rearrange("one (kt p) -> (one p) kt", p=128))
        for a in range(2):
            rwkv_region(a, XinIO, a)

        nc.gpsimd.collective_compute(
            "AllReduce", ALU.add, replica_groups=t["RG"],
            ins=[t["hpb"][:].opt()], outs=[t["hpar"][:].opt()])
        nc.sync.dma_start(t["hsum"][:], t["hpar"][:])

        # ---- head: motor -> pi, facets, u ----
        wfpool = ctx.enter_context(tc.tile_pool(name="wf", bufs=1))
        WfC = wfpool.tile([128, KF, KT, 128], f32)
        nc.sync.dma_start(WfC[:], t["WfC"][:].transpose([2, 0, 1, 3]))
        hs1 = sb.tile([128, KT], f32)
        nc.sync.dma_start(
            hs1[:],
            t["hpar"][1:2, :].rearrange("one (kt p) -> (one p) kt", p=128))
        motor = sb.tile([128, KT], f32)
        nc.vector.tensor_tensor(motor[:], hs1[:], XinIO[:, :, 1], op=ALU.add)

        pi_ps = rwk_ps.tile([128, NVT], f32, tag="rwk")
        for kt in range(KT):
            nc.tensor.matmul(pi_ps[0:KF, 0:1], wpiT[:, kt, :], motor[:, kt:kt+1],
                             start=(kt == 0), stop=(kt == KT - 1))
        pi_col = ppool.tile([KF, 1], f32, tag="picol")
        nc.vector.tensor_copy(pi_col[:], pi_ps[0:KF, 0:1])
        prow = ppool.tile([1, KF], f32, tag="prow")
        nc.sync.dma_start(prow[:], pi_col[:])
        pmax = ppool.tile([1, 1], f32, tag="pmax")
        nc.vector.tensor_reduce(pmax[:], prow[:], axis=AX.X, op=ALU.max)
        pmaxn = ppool.tile([1, 1], f32, tag="pmaxn")
        nc.vector.tensor_scalar(out=pmaxn[:], in0=pmax[:], scalar1=-1.0,
                                scalar2=None, op0=ALU.mult)
        pie = ppool.tile([1, KF], f32, tag="pie")
        pis = ppool.tile([1, 1], f32, tag="pis")
        nc.scalar.activation(pie[:], prow[:], AF.Exp, bias=pmaxn[:], accum_out=pis[:])
        pinv = ppool.tile([1, 1], f32, tag="pinv")
        nc.vector.reciprocal(pinv[:], pis[:])
        pirow = ppool.tile([1, KF], f32, tag="pirow")
        nc.vector.tensor_scalar(out=pirow[:], in0=pie[:], scalar1=pinv[:],
                                scalar2=None, op0=ALU.mult)

        F_sb = sb.tile([128, KF], f32)
        for kf in range(KF):
            f_ps = rwk_ps.tile([128, NVT], f32, tag="rwk")
            for kt in range(KT):
                nc.tensor.matmul(f_ps[:, 0:1], WfC[:, kf, kt, :], motor[:, kt:kt+1],
                                 start=(kt == 0), stop=(kt == KT - 1))
            nc.scalar.activation(F_sb[:, kf:kf+1], f_ps[:, 0:1], AF.Tanh)
        nc.sync.dma_start(t["pib"][:], pirow[:])
        pib_sb = sb.tile([128, KF], f32)
        nc.sync.dma_start(pib_sb[:], t["pib"][:].unsqueeze(0).to_broadcast([128, KF]))
        uprod = ppool.tile([128, KF], f32, tag="uprod")
        nc.vector.tensor_tensor(uprod[:], F_sb[:], pib_sb[:], op=ALU.mult)
        u_sb = ppool.tile([128, 1], f32, tag="usb")
        nc.vector.tensor_reduce(u_sb[:], uprod[:], axis=AX.X, op=ALU.add)
        nc.sync.dma_start(t["ub"][:], u_sb[:])
        nc.sync.dma_start(t["dbg_u"][:], u_sb[:])
        nc.sync.dma_start(t["dbg_pi"][:], pirow[:])
        nc.sync.dma_start(t["dbg_F"][:], F_sb[:])
        nc.sync.dma_start(
            t["dbg_motor"][:].rearrange("(kt p) -> p kt", p=128), motor[:])
        nc.gpsimd.collective_compute(
            "AllGather", ALU.bypass, replica_groups=t["RG"],
            ins=[t["ub"][:].opt()], outs=[t["uag"][:].opt()])
        uT = sb.tile([128, KT], f32)
        nc.sync.dma_start(
            uT[:], t["uag"][:].rearrange("(kt p) -> p kt", p=128))

        # ---- logits ----
        Lg = sb.tile([128, NVT], f32)
        lg_ps = rwk_ps.tile([128, NVT], f32, tag="rwk")
        with tc.tile_pool(name="wout", bufs=24) as wopool:
            for vt in range(NVT):
                for kt in range(KT):
                    wt_o = wopool.tile([128, 128], f32, tag="stripe")
                    nc.sync.dma_start(
                        wt_o[:], t["WoutC"][kt*128:(kt+1)*128,
                                            vt*128:(vt+1)*128])
                    nc.tensor.matmul(lg_ps[:, vt:vt+1], wt_o[:],
                                     uT[:, kt:kt+1],
                                     start=(kt == 0), stop=(kt == KT - 1))
        nc.vector.tensor_copy(Lg[:], lg_ps[:])
        nc.sync.dma_start(
            t["logits"][:].rearrange("(vt p) -> p vt", p=128),
            Lg[:])


def _host_prep(inputs):
    f = np.float32
    tokens = np.asarray(inputs["tokens"])
    focus = np.asarray(inputs["focus_map"], f)
    H_prev = np.asarray(inputs["H_prev"], f)
    pos = np.stack([np.arange(T, dtype=f), np.arange(T, dtype=f) / (T + 1e-9)], -1)
    x = np.asarray(inputs["embed_W"], f)[tokens] + pos @ np.asarray(inputs["W_pos"], f)
    fsm = np.exp(focus - focus.max()); fsm = (fsm / fsm.sum()).astype(f)

    sc = H_prev @ np.asarray(inputs["w_gate"], f)
    top = np.argsort(-sc, kind="stable")[:KACT]
    reg_mask = np.zeros(R, bool); reg_mask[top] = True; reg_mask[[IO_S, IO_M]] = True
    active = np.where(reg_mask)[0]
    A = len(active)

    Hm = np.where(np.asarray(inputs["reg_mask_prev"])[:, None], H_prev, 0.0).astype(f)
    nbr = np.asarray(inputs["nbr_idx"])
    rc = np.asarray(inputs["reg_coords"], f)
    dist = np.linalg.norm(rc[:, None, :] - rc[nbr], axis=-1)
    w = np.exp(-dist - (-dist).max(1, keepdims=True)); w = (w / w.sum(1, keepdims=True)).astype(f)
    Magg = np.einsum('rn,rnd->rd', w, Hm[nbr])
    M_act = (Magg[active] @ np.asarray(inputs["W_route"], f))
    gamma = np.asarray(inputs["gamma"], f)

    XinN = np.zeros((A, D), f)
    for a, r in enumerate(active):
        if r not in (IO_S, IO_M):
            v = M_act[a]
            XinN[a] = v * gamma * (1.0 / np.sqrt(np.mean(v * v) + 1e-6))
    spos = f(float(inputs["step_k"]) / float(max(1, KINNER - 1)))
    return dict(x=x.astype(f), fsm=fsm, reg_mask=reg_mask, active=active, A=A,
                M_act=M_act, XinN=XinN, spos=spos, gamma=gamma)


def kernel(**inputs):
    f = np.float32
    hp = _host_prep(inputs)
    A, active = hp["A"], hp["active"]
    x, fsm = hp["x"], hp["fsm"]

    Wr = np.asarray(inputs["Wr"], f); Wv = np.asarray(inputs["Wv"], f)
    Wo = np.asarray(inputs["Wo"], f); b_step = np.asarray(inputs["b_step"], f)
    Wf = np.asarray(inputs["Wf"], f); w_pi = np.asarray(inputs["w_pi"], f)
    W_out = np.asarray(inputs["W_out"], f)
    Wout_pad = np.zeros((D, VPAD), f); Wout_pad[:, :V] = W_out

    XinNT = np.ascontiguousarray(
        hp["XinN"].T.reshape(KT, 128, A).transpose(1, 0, 2))

    common = {
        "Wq": np.asarray(inputs["Wq"], f) / np.sqrt(float(HD)),
        "Wk": np.asarray(inputs["Wk"], f),
        "Wv_at": np.asarray(inputs["Wv_attn"], f),
        "Wo_at": np.asarray(inputs["Wo_attn"], f),
        "ones": np.ones((128, 128), f), "ident": np.eye(128, dtype=f),
        "onz": np.concatenate([np.ones((1, 128), f), np.zeros((1, 128), f)]),
        "gammaR": hp["gamma"].reshape(1, D),
        "MioR": np.ascontiguousarray(hp["M_act"][0:2]),
        "XinNT": XinNT,
        "wpiT": np.ascontiguousarray(w_pi.T),
    }
    in_maps = []
    for c in range(NCORES):
        esl = slice(c * EC, (c + 1) * EC)
        m = dict(common)
        m["xT"] = np.ascontiguousarray(x[c*TQ:(c+1)*TQ].T)
        m["fsm"] = np.concatenate([fsm[c*TQ:(c+1)*TQ].reshape(1, TQ),
                                   np.zeros((1, TQ), f)])
        m["WrC"] = np.ascontiguousarray(
            Wr[active][:, :, esl].reshape(A, KT, 128, EC))
        m["WvC"] = np.ascontiguousarray(
            Wv[active][:, :, esl].reshape(A, KT, 128, EC))
        m["WoC"] = np.ascontiguousarray(
            Wo[active][:, esl, :].transpose(0, 2, 1).reshape(A, KT, 128, EC)
            .transpose(0, 1, 3, 2))
        m["bstepC"] = np.ascontiguousarray((hp["spos"] * b_step[active][:, esl]).T)
        m["WfC"] = np.ascontiguousarray(Wf[:, :, esl].reshape(KF, KT, 128, EC))
        m["WoutC"] = np.ascontiguousarray(Wout_pad[:, c*VC:(c+1)*VC])
        in_maps.append(m)

    if A not in _prog_cache:
        _prog_cache[A] = _build(A)
    nc = _prog_cache[A]
    res = run_bass_kernel_spmd(nc, in_maps, list(range(NCORES)))
    global LAST_RES
    LAST_RES = res

    logits_pad = np.concatenate([res.results[c]["logits"] for c in range(NCORES)])
    logits = logits_pad[:V]
    sensor = res.results[0]["sensor"]
    hsum = res.results[0]["hsum"]

    gamma = hp["gamma"]
    Xin = np.zeros((A, D), f)
    for a, r in enumerate(active):
        v = hp["M_act"][a] + (sensor if r in (IO_S, IO_M) else 0.0)
        Xin[a] = v * gamma * (1.0 / np.sqrt(np.mean(v * v) + 1e-6))
    H_act = hsum + Xin
    H_cur = np.zeros((R, D), f); H_cur[active] = H_act
    reg_mask = hp["reg_mask"]
    motor = H_act[1]
    ws = H_act.sum(0) / max(int(reg_mask.sum()), 1)
    rtd = np.float32(motor @ np.asarray(inputs["w_rtd"], f))
    return (H_cur.astype(f), reg_mask, logits.astype(f), rtd,
            ws.astype(f), motor.astype(f))


# revision 21
# speedup vs baseline: 6284.4791x; 6284.4791x over previous
"""Trainium2 Bass kernel for nn_CortexReasoner (moe_routing).

Sharding across 8 NeuronCores:
  - Attention: T=4096 tokens sharded 512/core; K,V all-gathered (one fused AG).
  - Scores computed in [u, t] layout (u on partitions) so softmax'd probs feed
    the PV matmul directly with no transposes; per-row max via PE-transpose of
    the running column-max + exp applied straight out of PSUM with a
    (-max)-broadcast PSUM init matmul.
  - Gate/top-k/router aggregation are pure functions of the inputs -> host.
    RWKV step computed ONLY for active regions (<=10 of 32), e-dim sharded
    128/core; partial h AllReduced.
  - Output head: Wf e-sharded, W_out vocab-sharded 6400/core.
Matmuls run as float32r (hw-verified ~1.5e-4 relmax, full rate at N>=256).
"""
import sys
sys.path.insert(0, "/opt/trn_rl_repo")

import numpy as np
import concourse.bass as bass
import concourse.bacc as bacc
import concourse.tile as tile
import concourse.mybir as mybir
from concourse.bass_utils import run_bass_kernel_spmd

R, D, V, T, NBR = 32, 1024, 50257, 4096, 8
KF, NH, KACT, KINNER = 8, 4, 8, 8
IO_S, IO_M = 0, 1
HD = D // NH            # 256
NCORES = 8
TQ = T // NCORES        # 512 q rows per core
KT = D // 128           # 8 contraction chunks
NU = T // 128           # 32 u-tiles
VPAD = 51200
VC = VPAD // NCORES     # 6400
NVT = VC // 128         # 50
EC = D // NCORES        # 128 (e-chunk per core)
KVN = D * TQ

f32 = mybir.dt.float32
f32r = mybir.dt.float32r
AF = mybir.ActivationFunctionType
ALU = mybir.AluOpType
AX = mybir.AxisListType

_prog_cache = {}


def _build(A: int):
    nc = bacc.Bacc("TRN2", target_bir_lowering=False, debug=False,
                   num_devices=NCORES)
    t = {}

    def din(name, shape, dt=f32r):
        t[name] = nc.dram_tensor(name, shape, dt, kind="ExternalInput")

    din("xT", [D, TQ]); din("fsm", [2, TQ]); din("onz", [2, 128])
    din("Wq", [D, D]); din("Wk", [D, D]); din("Wv_at", [D, D]); din("Wo_at", [D, D])
    din("ones", [128, 128]); din("ident", [128, 128], f32)
    din("gammaR", [1, D], f32); din("MioR", [2, D], f32)
    din("XinNT", [128, KT, A], f32)
    din("WrC", [A, KT, 128, 128], f32); din("WvC", [A, KT, 128, 128], f32)
    din("WoC", [A, KT, 128, 128], f32); din("bstepC", [128, A], f32)
    din("wpiT", [D, KF], f32); din("WfC", [KF, KT, 128, 128], f32)
    din("WoutC", [D, VC], f32)

    t["logits"] = nc.dram_tensor("logits", [VC], f32, kind="ExternalOutput")
    t["sensor"] = nc.dram_tensor("sensor", [D], f32, kind="ExternalOutput")
    t["hsum"] = nc.dram_tensor("hsum", [A, D], f32, kind="ExternalOutput")
    t["dbg_pi"] = nc.dram_tensor("dbg_pi", [KF], f32, kind="ExternalOutput")
    t["dbg_u"] = nc.dram_tensor("dbg_u", [EC], f32, kind="ExternalOutput")
    t["dbg_motor"] = nc.dram_tensor("dbg_motor", [D], f32, kind="ExternalOutput")
    t["dbg_F"] = nc.dram_tensor("dbg_F", [128, KF], f32, kind="ExternalOutput")

    t["kvb"] = nc.dram_tensor("kv_bounce", [2 * KVN], f32r)
    t["kvag"] = nc.dram_tensor("kv_ag", [NCORES, 2 * KVN], f32r, addr_space="Shared")
    t["senb"] = nc.dram_tensor("sens_bounce", [D], f32)
    t["senar"] = nc.dram_tensor("sens_ar", [D], f32, addr_space="Shared")
    t["hpb"] = nc.dram_tensor("hp_bounce", [A, D], f32)
    t["hpar"] = nc.dram_tensor("hp_ar", [A, D], f32, addr_space="Shared")
    t["ub"] = nc.dram_tensor("u_bounce", [EC], f32)
    t["uag"] = nc.dram_tensor("u_ag", [D], f32, addr_space="Shared")
    t["xiob"] = nc.dram_tensor("xio_bounce", [2, D], f32)
    t["pib"] = nc.dram_tensor("pi_bounce", [KF], f32)
    t["RG"] = [list(range(NCORES))]

    with tile.TileContext(nc) as tc:
        _emit(nc, tc, A, t)
    nc.compile()
    return nc


def _emit(nc, tc, A, t):
    from contextlib import ExitStack
    ctx = ExitStack()
    with ctx:
        sb = ctx.enter_context(tc.tile_pool(name="sb", bufs=1))
        wpool = ctx.enter_context(tc.tile_pool(name="w", bufs=4))
        big_ps = ctx.enter_context(tc.tile_pool(name="bigps", bufs=2, space="PSUM"))
        rwk_ps = ctx.enter_context(tc.tile_pool(name="rwkps", bufs=2, space="PSUM"))
        ppool = ctx.enter_context(tc.tile_pool(name="p", bufs=2))
        pp3 = ctx.enter_context(tc.tile_pool(name="pp3", bufs=3))
        rowp = ctx.enter_context(tc.tile_pool(name="rowp", bufs=1))
        rwk_w = ctx.enter_context(tc.tile_pool(name="rwkw", bufs=4))

        # ---- constants / small loads ----
        ones = sb.tile([128, 128], f32r)
        nc.sync.dma_start(ones[:], t["ones"][:])
        onz = sb.tile([2, 128], f32r)
        nc.sync.dma_start(onz[:], t["onz"][:])
        ident = sb.tile([128, 128], f32)
        nc.sync.dma_start(ident[:], t["ident"][:])
        fsm = sb.tile([2, TQ], f32r)
        nc.sync.dma_start(fsm[:], t["fsm"][:])
        xT = sb.tile([128, KT, TQ], f32r)
        nc.sync.dma_start(xT[:], t["xT"][:].rearrange("(kt p) t -> p kt t", p=128))
        gamma = sb.tile([1, D], f32)
        nc.sync.dma_start(gamma[:], t["gammaR"][:])
        Mio = sb.tile([1, 2 * D], f32)
        nc.sync.dma_start(Mio[:], t["MioR"][:].rearrange("r d -> (r d)").unsqueeze(0))
        XinN = sb.tile([128, KT, A], f32)
        nc.sync.dma_start(XinN[:], t["XinNT"][:])
        bstep = sb.tile([128, A], f32)
        nc.sync.dma_start(bstep[:], t["bstepC"][:])
        wpiT = sb.tile([128, KT, KF], f32)
        nc.sync.dma_start(wpiT[:], t["wpiT"][:].rearrange("(kt p) k -> p kt k", p=128))

        hT_sb = sb.tile([128, A, KT], f32)

        def rwkv_region(a, xin_tile, xin_col):
            rhs = xin_tile[:, :, xin_col]
            rg_ps = rwk_ps.tile([128, NVT], f32, tag="rwk")
            for kt in range(KT):
                wr = rwk_w.tile([128, 128], f32, tag="wr")
                nc.sync.dma_start(wr[:], t["WrC"][a, kt])
                nc.tensor.matmul(rg_ps[:, 0:1], wr[:], rhs[:, kt:kt+1],
                                 start=(kt == 0), stop=(kt == KT - 1))
            vv_ps = rwk_ps.tile([128, NVT], f32, tag="rwk")
            for kt in range(KT):
                wv = rwk_w.tile([128, 128], f32, tag="wv")
                nc.sync.dma_start(wv[:], t["WvC"][a, kt])
                nc.tensor.matmul(vv_ps[:, 0:1], wv[:], rhs[:, kt:kt+1],
                                 start=(kt == 0), stop=(kt == KT - 1))
            rg = ppool.tile([128, 1], f32, tag="rg")
            nc.scalar.activation(rg[:], rg_ps[:, 0:1], AF.Sigmoid,
                                 bias=bstep[:, a:a+1])
            rgvv = ppool.tile([128, 1], f32, tag="rgvv")
            nc.vector.tensor_tensor(rgvv[:], rg[:], vv_ps[:, 0:1], op=ALU.mult)
            h_ps = rwk_ps.tile([128, NVT], f32, tag="rwk")
            for mt in range(KT):
                wo = rwk_w.tile([128, 128], f32, tag="wo")
                nc.sync.dma_start(wo[:], t["WoC"][a, mt])
                nc.tensor.matmul(h_ps[:, mt:mt+1], wo[:], rgvv[:],
                                 start=True, stop=True)
            nc.vector.tensor_copy(hT_sb[:, a, :], h_ps[:, 0:KT])
            nc.sync.dma_start(
                t["hpb"][a:a+1, :].rearrange("one (kt p) -> (one p) kt", p=128),
                hT_sb[:, a, :])

        for a in range(2, A):
            rwkv_region(a, XinN, a)

        # =========== attention projections ===========
        qT = sb.tile([128, KT, TQ], f32r)
        kvctx = ExitStack()
        kvpool = kvctx.enter_context(tc.tile_pool(name="kv", bufs=1))
        if True:
            kT_t = kvpool.tile([128, KT, TQ], f32r, tag="kh", name="kT_t")
            for (W_n, dst) in (("Wk", kT_t), ("Wq", qT)):
                for m in range(KT):
                    ps = big_ps.tile([128, TQ], f32, tag="big")
                    for kt in range(KT):
                        wt = wpool.tile([128, 128], f32r, tag="wt")
                        nc.sync.dma_start(
                            wt[:], t[W_n][kt*128:(kt+1)*128, m*128:(m+1)*128])
                        nc.tensor.matmul(ps[:], wt[:], xT[:, kt, :],
                                         start=(kt == 0), stop=(kt == KT - 1))
                    nc.vector.tensor_copy(dst[:, m, :], ps[:])
            v_t = kvpool.tile([128, 4, D], f32r, tag="vh", name="v_t")
            for tm in range(4):
                for nh in range(2):
                    ps = big_ps.tile([128, 512], f32, tag="big")
                    for kt in range(KT):
                        wt = wpool.tile([128, 512], f32r, tag="wtv")
                        nc.sync.dma_start(wt[:], t["Wv_at"][kt*128:(kt+1)*128,
                                                            nh*512:(nh+1)*512])
                        nc.tensor.matmul(ps[:], xT[:, kt, tm*128:(tm+1)*128], wt[:],
                                         start=(kt == 0), stop=(kt == KT - 1))
                    nc.vector.tensor_copy(v_t[:, tm, nh*512:(nh+1)*512], ps[:])
            nc.sync.dma_start(
                t["kvb"][0:KVN].rearrange("(m p tt) -> p m tt", m=KT, p=128),
                kT_t[:])
            nc.sync.dma_start(
                t["kvb"][KVN:2*KVN].rearrange("(tm p d) -> p tm d", tm=4, p=128),
                v_t[:])
        nc.gpsimd.collective_compute(
            "AllGather", ALU.bypass, replica_groups=t["RG"],
            ins=[t["kvb"][:].opt()], outs=[t["kvag"][:].opt()])

        # =========== attention heads ===========
        oT = sb.tile([128, KT, TQ], f32r)
        with tc.tile_pool(name="attacc", bufs=1, space="PSUM") as att_ps:
            for h in range(NH):
                Kh = kvpool.tile([128, 2, NCORES, TQ], f32r, tag="kh")
                Vh = kvpool.tile([128, NU, HD], f32r, tag="vh")
                for cb in range(NCORES):
                    kblk = t["kvag"][cb, 0:KVN].rearrange(
                        "(d tt) -> d tt", d=D)[h*HD:(h+1)*HD, :]
                    nc.sync.dma_start(
                        Kh[:, :, cb, :],
                        kblk.rearrange("(dd p) tt -> p dd tt", p=128))
                    vblk = t["kvag"][cb, KVN:2*KVN].rearrange(
                        "(u d) -> u d", d=D)[:, h*HD:(h+1)*HD]
                    nc.sync.dma_start(
                        Vh[:, cb*4:(cb+1)*4, :],
                        vblk.rearrange("(tl p) d -> p tl d", p=128))
                qh0 = qT[:, 2*h, :]
                qh1 = qT[:, 2*h+1, :]

                def kslice(dd, ut):
                    return Kh[:, dd, ut // 4, (ut % 4)*128:(ut % 4 + 1)*128]

                # pass 1: scores + running col-max
                smax = ppool.tile([128, TQ], f32, tag="smax", bufs=1)
                for ut in range(NU):
                    s_ps = big_ps.tile([128, TQ], f32, tag="big")
                    nc.tensor.matmul(s_ps[:], kslice(0, ut), qh0, start=True, stop=False)
                    nc.tensor.matmul(s_ps[:], kslice(1, ut), qh1, start=False, stop=True)
                    if ut == 0:
                        nc.vector.tensor_copy(smax[:], s_ps[:])
                    else:
                        nc.vector.tensor_tensor(smax[:], smax[:], s_ps[:], op=ALU.max)
                mrow = ppool.tile([2, TQ], f32, tag="mrow", bufs=1)
                for tck in range(4):
                    tr = big_ps.tile([128, 128], f32, tag="big")
                    nc.tensor.transpose(tr[:], smax[:, tck*128:(tck+1)*128], ident[:])
                    mcol = ppool.tile([128, 1], f32, tag="mcol")
                    nc.vector.tensor_reduce(mcol[:], tr[:], axis=AX.X, op=ALU.max)
                    nc.sync.dma_start(mrow[0:1, tck*128:(tck+1)*128], mcol[:])
                    nc.sync.dma_start(mrow[1:2, tck*128:(tck+1)*128], mcol[:])
                mneg = ppool.tile([2, TQ], f32r, tag="mneg", bufs=1)
                nc.vector.tensor_scalar(out=mneg[:], in0=mrow[:], scalar1=-1.0,
                                        scalar2=None, op0=ALU.mult)
                # pass 2
                l_ps = att_ps.tile([2, TQ], f32, tag="l")
                o_ps0 = att_ps.tile([128, TQ], f32, tag="o0")
                o_ps1 = att_ps.tile([128, TQ], f32, tag="o1")
                for ut in range(NU):
                    s_ps = big_ps.tile([128, TQ], f32, tag="big")
                    nc.tensor.matmul(s_ps[:], onz[:], mneg[:], start=True, stop=False)
                    nc.tensor.matmul(s_ps[:], kslice(0, ut), qh0, start=False, stop=False)
                    nc.tensor.matmul(s_ps[:], kslice(1, ut), qh1, start=False, stop=True)
                    p = pp3.tile([128, TQ], f32r, tag="p")
                    nc.scalar.activation(p[:], s_ps[:], AF.Exp)
                    nc.tensor.matmul(l_ps[:], ones[:, 0:2], p[:],
                                     start=(ut == 0), stop=(ut == NU - 1))
                    nc.tensor.matmul(o_ps0[:], Vh[:, ut, 0:128], p[:],
                                     start=(ut == 0), stop=(ut == NU - 1))
                    nc.tensor.matmul(o_ps1[:], Vh[:, ut, 128:256], p[:],
                                     start=(ut == 0), stop=(ut == NU - 1))
                linv = ppool.tile([2, TQ], f32r, tag="linv")
                with nc.allow_low_precision(reason="f32r rounding feeds matmul"):
                    nc.vector.reciprocal(linv[:], l_ps[:])
                lbc_ps = big_ps.tile([128, TQ], f32, tag="big")
                nc.tensor.matmul(lbc_ps[:], onz[:], linv[:], start=True, stop=True)
                lbc = ppool.tile([128, TQ], f32, tag="lbcs", bufs=1)
                nc.vector.tensor_copy(lbc[:], lbc_ps[:])
                nc.vector.tensor_tensor(oT[:, 2*h, :], o_ps0[:], lbc[:], op=ALU.mult)
                nc.vector.tensor_tensor(oT[:, 2*h+1, :], o_ps1[:], lbc[:], op=ALU.mult)

        kvctx.close()
        # ---- o-proj + residual + sensor partial ----
        fsm_ps = big_ps.tile([128, TQ], f32, tag="big")
        nc.tensor.matmul(fsm_ps[:], onz[:], fsm[:], start=True, stop=True)
        fsm_bc = sb.tile([128, TQ], f32)
        nc.vector.tensor_copy(fsm_bc[:], fsm_ps[:])
        sens = sb.tile([128, KT], f32)
        for m in range(KT):
            ps = big_ps.tile([128, TQ], f32, tag="big")
            for kt in range(KT):
                wt = wpool.tile([128, 128], f32r, tag="wt")
                nc.sync.dma_start(wt[:], t["Wo_at"][kt*128:(kt+1)*128, m*128:(m+1)*128])
                nc.tensor.matmul(ps[:], wt[:], oT[:, kt, :],
                                 start=(kt == 0), stop=(kt == KT - 1))
            mix = ppool.tile([128, TQ], f32, tag="mix")
            nc.vector.tensor_tensor(mix[:], ps[:], xT[:, m, :], op=ALU.add)
            mixf = ppool.tile([128, TQ], f32, tag="mixf")
            nc.vector.tensor_tensor(mixf[:], mix[:], fsm_bc[:], op=ALU.mult)
            nc.vector.tensor_reduce(sens[:, m:m+1], mixf[:], axis=AX.X, op=ALU.add)
        nc.sync.dma_start(
            t["senb"][:].rearrange("(m p) -> p m", p=128),
            sens[:])
        nc.gpsimd.collective_compute(
            "AllReduce", ALU.add, replica_groups=t["RG"],
            ins=[t["senb"][:].opt()], outs=[t["senar"][:].opt()])
        nc.sync.dma_start(t["sensor"][:], t["senar"][:])

        # ---- Xin rows 0,1 ----
        sens_row = sb.tile([1, D], f32)
        nc.sync.dma_start(sens_row[:], t["senar"][:].unsqueeze(0))
        eps_t = sb.tile([1, 1], f32)
        nc.vector.memset(eps_t[:], 1e-6)
        XinIO = sb.tile([128, KT, 2], f32)
        for r in range(2):
            y = rowp.tile([1, D], f32, tag="y")
            nc.vector.tensor_tensor(y[:], Mio[0:1, r*D:(r+1)*D], sens_row[:], op=ALU.add)
            ysq = rowp.tile([1, D], f32, tag="ysq")
            ssq = rowp.tile([1, 1], f32, tag="ssq")
            nc.scalar.activation(ysq[:], y[:], AF.Square, accum_out=ssq[:])
            sd = rowp.tile([1, 1], f32, tag="sd")
            nc.scalar.activation(sd[:], ssq[:], AF.Sqrt, bias=eps_t[:], scale=1.0 / D)
            rinv = rowp.tile([1, 1], f32, tag="rinv")
            nc.vector.reciprocal(rinv[:], sd[:])
            yn = rowp.tile([1, D], f32, tag="yn")
            nc.vector.tensor_scalar(out=yn[:], in0=y[:], scalar1=rinv[:],
                                    scalar2=None, op0=ALU.mult)
            yg = rowp.tile([1, D], f32, tag="yg")
            nc.vector.tensor_tensor(yg[:], yn[:], gamma[:], op=ALU.mult)
            nc.sync.dma_start(t["xiob"][r:r+1, :], yg[:])
            nc.sync.dma_start(
                XinIO[:, :, r],
                t["xiob"][r:r+1, :].rearrange("one (kt p) -> (one p) kt", p=128))
        for a in range(2):
            rwkv_region(a, XinIO, a)

        nc.gpsimd.collective_compute(
            "AllReduce", ALU.add, replica_groups=t["RG"],
            ins=[t["hpb"][:].opt()], outs=[t["hpar"][:].opt()])
        nc.sync.dma_start(t["hsum"][:], t["hpar"][:])

        # ---- head: motor -> pi, facets, u ----
        wfpool = ctx.enter_context(tc.tile_pool(name="wf", bufs=1))
        WfC = wfpool.tile([128, KF, KT, 128], f32)
        nc.sync.dma_start(WfC[:], t["WfC"][:].transpose([2, 0, 1, 3]))
        hs1 = sb.tile([128, KT], f32)
        nc.sync.dma_start(
            hs1[:],
            t["hpar"][1:2, :].rearrange("one (kt p) -> (one p) kt", p=128))
        motor = sb.tile([128, KT], f32)
        nc.vector.tensor_tensor(motor[:], hs1[:], XinIO[:, :, 1], op=ALU.add)

        pi_ps = rwk_ps.tile([128, NVT], f32, tag="rwk")
        for kt in range(KT):
            nc.tensor.matmul(pi_ps[0:KF, 0:1], wpiT[:, kt, :], motor[:, kt:kt+1],
                             start=(kt == 0), stop=(kt == KT - 1))
        pi_col = ppool.tile([KF, 1], f32, tag="picol")
        nc.vector.tensor_copy(pi_col[:], pi_ps[0:KF, 0:1])
        prow = ppool.tile([1, KF], f32, tag="prow")
        nc.sync.dma_start(prow[:], pi_col[:])
        pmax = ppool.tile([1, 1], f32, tag="pmax")
        nc.vector.tensor_reduce(pmax[:], prow[:], axis=AX.X, op=ALU.max)
        pmaxn = ppool.tile([1, 1], f32, tag="pmaxn")
        nc.vector.tensor_scalar(out=pmaxn[:], in0=pmax[:], scalar1=-1.0,
                                scalar2=None, op0=ALU.mult)
        pie = ppool.tile([1, KF], f32, tag="pie")
        pis = ppool.tile([1, 1], f32, tag="pis")
        nc.scalar.activation(pie[:], prow[:], AF.Exp, bias=pmaxn[:], accum_out=pis[:])
        pinv = ppool.tile([1, 1], f32, tag="pinv")
        nc.vector.reciprocal(pinv[:], pis[:])
        pirow = ppool.tile([1, KF], f32, tag="pirow")
        nc.vector.tensor_scalar(out=pirow[:], in0=pie[:], scalar1=pinv[:],
                                scalar2=None, op0=ALU.mult)

        F_sb = sb.tile([128, KF], f32)
        for kf in range(KF):
            f_ps = rwk_ps.tile([128, NVT], f32, tag="rwk")
            for kt in range(KT):
                nc.tensor.matmul(f_ps[:, 0:1], WfC[:, kf, kt, :], motor[:, kt:kt+1],
                                 start=(kt == 0), stop=(kt == KT - 1))
            nc.scalar.activation(F_sb[:, kf:kf+1], f_ps[:, 0:1], AF.Tanh)
        nc.sync.dma_start(t["pib"][:], pirow[:])
        pib_sb = sb.tile([128, KF], f32)
        nc.sync.dma_start(pib_sb[:], t["pib"][:].unsqueeze(0).to_broadcast([128, KF]))
        uprod = ppool.tile([128, KF], f32, tag="uprod")
        nc.vector.tensor_tensor(uprod[:], F_sb[:], pib_sb[:], op=ALU.mult)
        u_sb = ppool.tile([128, 1], f32, tag="usb")
        nc.vector.tensor_reduce(u_sb[:], uprod[:], axis=AX.X, op=ALU.add)
        nc.sync.dma_start(t["ub"][:], u_sb[:])
        nc.sync.dma_start(t["dbg_u"][:], u_sb[:])
        nc.sync.dma_start(t["dbg_pi"][:], pirow[:])
        nc.sync.dma_start(t["dbg_F"][:], F_sb[:])
        nc.sync.dma_start(
            t["dbg_motor"][:].rearrange("(kt p) -> p kt", p=128), motor[:])
        nc.gpsimd.collective_compute(
            "AllGather", ALU.bypass, replica_groups=t["RG"],
            ins=[t["ub"][:].opt()], outs=[t["uag"][:].opt()])
        uT = sb.tile([128, KT], f32)
        nc.sync.dma_start(
            uT[:], t["uag"][:].rearrange("(kt p) -> p kt", p=128))

        # ---- logits ----
        Lg = sb.tile([128, NVT], f32)
        lg_ps = rwk_ps.tile([128, NVT], f32, tag="rwk")
        with tc.tile_pool(name="wout", bufs=24) as wopool:
            for vt in range(NVT):
                for kt in range(KT):
                    wt_o = wopool.tile([128, 128], f32, tag="stripe")
                    nc.sync.dma_start(
                        wt_o[:], t["WoutC"][kt*128:(kt+1)*128,
                                            vt*128:(vt+1)*128])
                    nc.tensor.matmul(lg_ps[:, vt:vt+1], wt_o[:],
                                     uT[:, kt:kt+1],
                                     start=(kt == 0), stop=(kt == KT - 1))
        nc.vector.tensor_copy(Lg[:], lg_ps[:])
        nc.sync.dma_start(
            t["logits"][:].rearrange("(vt p) -> p vt", p=128),
            Lg[:])


def _host_prep(inputs):
    f = np.float32
    tokens = np.asarray(inputs["tokens"])
    focus = np.asarray(inputs["focus_map"], f)
    H_prev = np.asarray(inputs["H_prev"], f)
    pos = np.stack([np.arange(T, dtype=f), np.arange(T, dtype=f) / (T + 1e-9)], -1)
    x = np.asarray(inputs["embed_W"], f)[tokens] + pos @ np.asarray(inputs["W_pos"], f)
    fsm = np.exp(focus - focus.max()); fsm = (fsm / fsm.sum()).astype(f)

    sc = H_prev @ np.asarray(inputs["w_gate"], f)
    top = np.argsort(-sc, kind="stable")[:KACT]
    reg_mask = np.zeros(R, bool); reg_mask[top] = True; reg_mask[[IO_S, IO_M]] = True
    active = np.where(reg_mask)[0]
    A = len(active)

    Hm = np.where(np.asarray(inputs["reg_mask_prev"])[:, None], H_prev, 0.0).astype(f)
    nbr = np.asarray(inputs["nbr_idx"])
    rc = np.asarray(inputs["reg_coords"], f)
    dist = np.linalg.norm(rc[:, None, :] - rc[nbr], axis=-1)
    w = np.exp(-dist - (-dist).max(1, keepdims=True)); w = (w / w.sum(1, keepdims=True)).astype(f)
    Magg = np.einsum('rn,rnd->rd', w, Hm[nbr])
    M_act = (Magg[active] @ np.asarray(inputs["W_route"], f))
    gamma = np.asarray(inputs["gamma"], f)

    XinN = np.zeros((A, D), f)
    for a, r in enumerate(active):
        if r not in (IO_S, IO_M):
            v = M_act[a]
            XinN[a] = v * gamma * (1.0 / np.sqrt(np.mean(v * v) + 1e-6))
    spos = f(float(inputs["step_k"]) / float(max(1, KINNER - 1)))
    return dict(x=x.astype(f), fsm=fsm, reg_mask=reg_mask, active=active, A=A,
                M_act=M_act, XinN=XinN, spos=spos, gamma=gamma)


def kernel(**inputs):
    f = np.float32
    hp = _host_prep(inputs)
    A, active = hp["A"], hp["active"]
    x, fsm = hp["x"], hp["fsm"]

    Wr = np.asarray(inputs["Wr"], f); Wv = np.asarray(inputs["Wv"], f)
    Wo = np.asarray(inputs["Wo"], f); b_step = np.asarray(inputs["b_step"], f)
    Wf = np.asarray(inputs["Wf"], f); w_pi = np.asarray(inputs["w_pi"], f)
    W_out = np.asarray(inputs["W_out"], f)
    Wout_pad = np.zeros((D, VPAD), f); Wout_pad[:, :V] = W_out

    XinNT = np.ascontiguousarray(
        hp["XinN"].T.reshape(KT, 128, A).transpose(1, 0, 2))

    common = {
        "Wq": np.asarray(inputs["Wq"], f) / np.sqrt(float(HD)),
        "Wk": np.asarray(inputs["Wk"], f),
        "Wv_at": np.asarray(inputs["Wv_attn"], f),
        "Wo_at": np.asarray(inputs["Wo_attn"], f),
        "ones": np.ones((128, 128), f), "ident": np.eye(128, dtype=f),
        "onz": np.concatenate([np.ones((1, 128), f), np.zeros((1, 128), f)]),
        "gammaR": hp["gamma"].reshape(1, D),
        "MioR": np.ascontiguousarray(hp["M_act"][0:2]),
        "XinNT": XinNT,
        "wpiT": np.ascontiguousarray(w_pi.T),
    }
    in_maps = []
    for c in range(NCORES):
        esl = slice(c * EC, (c + 1) * EC)
        m = dict(common)
        m["xT"] = np.ascontiguousarray(x[c*TQ:(c+1)*TQ].T)
        m["fsm"] = np.concatenate([fsm[c*TQ:(c+1)*TQ].reshape(1, TQ),
                                   np.zeros((1, TQ), f)])
        m["WrC"] = np.ascontiguousarray(
            Wr[active][:, :, esl].reshape(A, KT, 128, EC))
        m["WvC"] = np.ascontiguousarray(
            Wv[active][:, :, esl].reshape(A, KT, 128, EC))
        m["WoC"] = np.ascontiguousarray(
            Wo[active][:, esl, :].transpose(0, 2, 1).reshape(A, KT, 128, EC)
            .transpose(0, 1, 3, 2))
        m["bstepC"] = np.ascontiguousarray((hp["spos"] * b_step[active][:, esl]).T)
        m["WfC"] = np.ascontiguousarray(Wf[:, :, esl].reshape(KF, KT, 128, EC))
        m["WoutC"] = np.ascontiguousarray(Wout_pad[:, c*VC:(c+1)*VC])
        in_maps.append(m)

    if A not in _prog_cache:
        _prog_cache[A] = _build(A)
    nc = _prog_cache[A]
    res = run_bass_kernel_spmd(nc, in_maps, list(range(NCORES)))
    global LAST_RES
    LAST_RES = res

    logits_pad = np.concatenate([res.results[c]["logits"] for c in range(NCORES)])
    logits = logits_pad[:V]
    sensor = res.results[0]["sensor"]
    hsum = res.results[0]["hsum"]

    gamma = hp["gamma"]
    Xin = np.zeros((A, D), f)
    for a, r in enumerate(active):
        v = hp["M_act"][a] + (sensor if r in (IO_S, IO_M) else 0.0)
        Xin[a] = v * gamma * (1.0 / np.sqrt(np.mean(v * v) + 1e-6))
    H_act = hsum + Xin
    H_cur = np.zeros((R, D), f); H_cur[active] = H_act
    reg_mask = hp["reg_mask"]
    motor = H_act[1]
    ws = H_act.sum(0) / max(int(reg_mask.sum()), 1)
    rtd = np.float32(motor @ np.asarray(inputs["w_rtd"], f))
    return (H_cur.astype(f), reg_mask, logits.astype(f), rtd,
            ws.astype(f), motor.astype(f))


# revision 25
# speedup vs baseline: 6478.6606x; 1.0309x over previous
"""Trainium2 Bass kernel for nn_CortexReasoner (moe_routing).

Sharding across 8 NeuronCores:
  - Attention: T=4096 tokens sharded 512/core; K,V all-gathered (one fused AG).
  - Scores computed in [u, t] layout (u on partitions) so softmax'd probs feed
    the PV matmul directly with no transposes; per-row max via PE-transpose of
    the running column-max + exp applied straight out of PSUM with a
    (-max)-broadcast PSUM init matmul.
  - Gate/top-k/router aggregation are pure functions of the inputs -> host.
    RWKV step computed ONLY for active regions (<=10 of 32), e-dim sharded
    128/core; partial h AllReduced.
  - Output head: Wf e-sharded, W_out vocab-sharded 6400/core.
Matmuls run as float32r (hw-verified ~1.5e-4 relmax, full rate at N>=256).
"""
import sys
sys.path.insert(0, "/opt/trn_rl_repo")

import numpy as np
import concourse.bass as bass
import concourse.bacc as bacc
import concourse.tile as tile
import concourse.mybir as mybir
from concourse.bass_utils import run_bass_kernel_spmd

R, D, V, T, NBR = 32, 1024, 50257, 4096, 8
KF, NH, KACT, KINNER = 8, 4, 8, 8
IO_S, IO_M = 0, 1
HD = D // NH            # 256
NCORES = 8
TQ = T // NCORES        # 512 q rows per core
KT = D // 128           # 8 contraction chunks
NU = T // 128           # 32 u-tiles
VPAD = 51200
VC = VPAD // NCORES     # 6400
NVT = VC // 128         # 50
EC = D // NCORES        # 128 (e-chunk per core)
KVN = D * TQ

f32 = mybir.dt.float32
f32r = mybir.dt.float32r
AF = mybir.ActivationFunctionType
ALU = mybir.AluOpType
AX = mybir.AxisListType

_prog_cache = {}


def _build(A: int):
    nc = bacc.Bacc("TRN2", target_bir_lowering=False, debug=False,
                   num_devices=NCORES)
    t = {}

    def din(name, shape, dt=f32r):
        t[name] = nc.dram_tensor(name, shape, dt, kind="ExternalInput")

    din("xT", [D, TQ]); din("fsm", [2, TQ]); din("onz", [2, 128])
    din("Wq", [D, D]); din("Wk", [D, D]); din("Wv_at", [D, D]); din("Wo_at", [D, D])
    din("ones", [128, 128]); din("ident", [128, 128], f32)
    din("gammaR", [1, D], f32); din("MioR", [2, D], f32)
    din("XinNT", [128, KT, A], f32)
    din("WrC", [A, KT, 128, 128], f32); din("WvC", [A, KT, 128, 128], f32)
    din("WoC", [A, KT, 128, 128], f32); din("bstepC", [128, A], f32)
    din("wpiT", [D, KF], f32); din("WfC", [KF, KT, 128, 128], f32)
    din("WoutC", [D, VC], f32)

    t["logits"] = nc.dram_tensor("logits", [VC], f32, kind="ExternalOutput")
    t["sensor"] = nc.dram_tensor("sensor", [D], f32, kind="ExternalOutput")
    t["hsum"] = nc.dram_tensor("hsum", [A, D], f32, kind="ExternalOutput")
    t["dbg_pi"] = nc.dram_tensor("dbg_pi", [KF], f32, kind="ExternalOutput")
    t["dbg_u"] = nc.dram_tensor("dbg_u", [EC], f32, kind="ExternalOutput")
    t["dbg_motor"] = nc.dram_tensor("dbg_motor", [D], f32, kind="ExternalOutput")
    t["dbg_F"] = nc.dram_tensor("dbg_F", [128, KF], f32, kind="ExternalOutput")

    t["kvb"] = nc.dram_tensor("kv_bounce", [2 * KVN], f32r)
    t["kvag"] = nc.dram_tensor("kv_ag", [NCORES, 2 * KVN], f32r, addr_space="Shared")
    t["senb"] = nc.dram_tensor("sens_bounce", [D], f32)
    t["senar"] = nc.dram_tensor("sens_ar", [D], f32, addr_space="Shared")
    t["hpb"] = nc.dram_tensor("hp_bounce", [A, D], f32)
    t["hpar"] = nc.dram_tensor("hp_ar", [A, D], f32, addr_space="Shared")
    t["ub"] = nc.dram_tensor("u_bounce", [EC], f32)
    t["uag"] = nc.dram_tensor("u_ag", [D], f32, addr_space="Shared")
    t["xiob"] = nc.dram_tensor("xio_bounce", [2, D], f32)
    t["pib"] = nc.dram_tensor("pi_bounce", [KF], f32)
    t["RG"] = [list(range(NCORES))]

    with tile.TileContext(nc) as tc:
        _emit(nc, tc, A, t)
    nc.compile()
    return nc


def _emit(nc, tc, A, t):
    from contextlib import ExitStack
    ctx = ExitStack()
    with ctx:
        sb = ctx.enter_context(tc.tile_pool(name="sb", bufs=1))
        wpool = ctx.enter_context(tc.tile_pool(name="w", bufs=4))
        big_ps = ctx.enter_context(tc.tile_pool(name="bigps", bufs=3, space="PSUM"))
        rwk_ps = ctx.enter_context(tc.tile_pool(name="rwkps", bufs=2, space="PSUM"))
        ppool = ctx.enter_context(tc.tile_pool(name="p", bufs=2))
        pp3 = ctx.enter_context(tc.tile_pool(name="pp3", bufs=3))
        rowp = ctx.enter_context(tc.tile_pool(name="rowp", bufs=1))
        rwk_w = ctx.enter_context(tc.tile_pool(name="rwkw", bufs=4))

        # ---- constants / small loads ----
        ones = sb.tile([128, 128], f32r)
        nc.sync.dma_start(ones[:], t["ones"][:])
        onz = sb.tile([2, 128], f32r)
        nc.sync.dma_start(onz[:], t["onz"][:])
        ident = sb.tile([128, 128], f32)
        nc.sync.dma_start(ident[:], t["ident"][:])
        fsm = sb.tile([2, TQ], f32r)
        nc.sync.dma_start(fsm[:], t["fsm"][:])
        xT = sb.tile([128, KT, TQ], f32r)
        nc.sync.dma_start(xT[:], t["xT"][:].rearrange("(kt p) t -> p kt t", p=128))
        gamma = sb.tile([1, D], f32)
        nc.sync.dma_start(gamma[:], t["gammaR"][:])
        Mio = sb.tile([1, 2 * D], f32)
        nc.sync.dma_start(Mio[:], t["MioR"][:].rearrange("r d -> (r d)").unsqueeze(0))
        XinN = sb.tile([128, KT, A], f32)
        nc.sync.dma_start(XinN[:], t["XinNT"][:])
        bstep = sb.tile([128, A], f32)
        nc.sync.dma_start(bstep[:], t["bstepC"][:])
        wpiT = sb.tile([128, KT, KF], f32)
        nc.sync.dma_start(wpiT[:], t["wpiT"][:].rearrange("(kt p) k -> p kt k", p=128))

        hT_sb = sb.tile([128, A, KT], f32)

        def rwkv_region(a, xin_tile, xin_col):
            rhs = xin_tile[:, :, xin_col]
            rg_ps = rwk_ps.tile([128, NVT], f32, tag="rwk")
            for kt in range(KT):
                wr = rwk_w.tile([128, 128], f32, tag="wr")
                nc.sync.dma_start(wr[:], t["WrC"][a, kt])
                nc.tensor.matmul(rg_ps[:, 0:1], wr[:], rhs[:, kt:kt+1],
                                 start=(kt == 0), stop=(kt == KT - 1))
            vv_ps = rwk_ps.tile([128, NVT], f32, tag="rwk")
            for kt in range(KT):
                wv = rwk_w.tile([128, 128], f32, tag="wv")
                nc.sync.dma_start(wv[:], t["WvC"][a, kt])
                nc.tensor.matmul(vv_ps[:, 0:1], wv[:], rhs[:, kt:kt+1],
                                 start=(kt == 0), stop=(kt == KT - 1))
            rg = ppool.tile([128, 1], f32, tag="rg")
            nc.scalar.activation(rg[:], rg_ps[:, 0:1], AF.Sigmoid,
                                 bias=bstep[:, a:a+1])
            rgvv = ppool.tile([128, 1], f32, tag="rgvv")
            nc.vector.tensor_tensor(rgvv[:], rg[:], vv_ps[:, 0:1], op=ALU.mult)
            h_ps = rwk_ps.tile([128, NVT], f32, tag="rwk")
            for mt in range(KT):
                wo = rwk_w.tile([128, 128], f32, tag="wo")
                nc.sync.dma_start(wo[:], t["WoC"][a, mt])
                nc.tensor.matmul(h_ps[:, mt:mt+1], wo[:], rgvv[:],
                                 start=True, stop=True)
            nc.vector.tensor_copy(hT_sb[:, a, :], h_ps[:, 0:KT])
            nc.sync.dma_start(
                t["hpb"][a:a+1, :].rearrange("one (kt p) -> (one p) kt", p=128),
                hT_sb[:, a, :])

        for a in range(2, A):
            rwkv_region(a, XinN, a)

        # =========== attention projections ===========
        qT = sb.tile([128, KT, TQ], f32r)
        kvctx = ExitStack()
        kvpool = kvctx.enter_context(tc.tile_pool(name="kv", bufs=1))
        if True:
            kT_t = kvpool.tile([128, KT, TQ], f32r, tag="kh", name="kT_t")
            for (W_n, dst) in (("Wk", kT_t), ("Wq", qT)):
                for m in range(KT):
                    ps = big_ps.tile([128, TQ], f32, tag="big")
                    for kt in range(KT):
                        wt = wpool.tile([128, 128], f32r, tag="wt")
                        nc.sync.dma_start(
                            wt[:], t[W_n][kt*128:(kt+1)*128, m*128:(m+1)*128])
                        nc.tensor.matmul(ps[:], wt[:], xT[:, kt, :],
                                         start=(kt == 0), stop=(kt == KT - 1))
                    nc.vector.tensor_copy(dst[:, m, :], ps[:])
            v_t = kvpool.tile([128, 4, D], f32r, tag="vh", name="v_t")
            for tm in range(4):
                for nh in range(2):
                    ps = big_ps.tile([128, 512], f32, tag="big")
                    for kt in range(KT):
                        wt = wpool.tile([128, 512], f32r, tag="wtv")
                        nc.sync.dma_start(wt[:], t["Wv_at"][kt*128:(kt+1)*128,
                                                            nh*512:(nh+1)*512])
                        nc.tensor.matmul(ps[:], xT[:, kt, tm*128:(tm+1)*128], wt[:],
                                         start=(kt == 0), stop=(kt == KT - 1))
                    nc.vector.tensor_copy(v_t[:, tm, nh*512:(nh+1)*512], ps[:])
            nc.sync.dma_start(
                t["kvb"][0:KVN].rearrange("(m p tt) -> p m tt", m=KT, p=128),
                kT_t[:])
            nc.sync.dma_start(
                t["kvb"][KVN:2*KVN].rearrange("(tm p d) -> p tm d", tm=4, p=128),
                v_t[:])
        nc.gpsimd.collective_compute(
            "AllGather", ALU.bypass, replica_groups=t["RG"],
            ins=[t["kvb"][:].opt()], outs=[t["kvag"][:].opt()])

        # =========== attention heads ===========
        oT = sb.tile([128, KT, TQ], f32r)
        with tc.tile_pool(name="attacc", bufs=1, space="PSUM") as att_ps:
            for h in range(NH):
                Kh = kvpool.tile([128, 2, NCORES, TQ], f32r, tag="kh")
                Vh = kvpool.tile([128, NU, HD], f32r, tag="vh")
                for cb in range(NCORES):
                    kblk = t["kvag"][cb, 0:KVN].rearrange(
                        "(d tt) -> d tt", d=D)[h*HD:(h+1)*HD, :]
                    nc.sync.dma_start(
                        Kh[:, :, cb, :],
                        kblk.rearrange("(dd p) tt -> p dd tt", p=128))
                    vblk = t["kvag"][cb, KVN:2*KVN].rearrange(
                        "(u d) -> u d", d=D)[:, h*HD:(h+1)*HD]
                    nc.sync.dma_start(
                        Vh[:, cb*4:(cb+1)*4, :],
                        vblk.rearrange("(tl p) d -> p tl d", p=128))
                qh0 = qT[:, 2*h, :]
                qh1 = qT[:, 2*h+1, :]

                def kslice(dd, ut):
                    return Kh[:, dd, ut // 4, (ut % 4)*128:(ut % 4 + 1)*128]

                # pass 1: scores + running col-max
                smax = ppool.tile([128, TQ], f32, tag="smax", bufs=1)
                for ut in range(NU):
                    s_ps = big_ps.tile([128, TQ], f32, tag="big")
                    nc.tensor.matmul(s_ps[:], kslice(0, ut), qh0, start=True, stop=False)
                    nc.tensor.matmul(s_ps[:], kslice(1, ut), qh1, start=False, stop=True)
                    if ut == 0:
                        nc.vector.tensor_copy(smax[:], s_ps[:])
                    else:
                        nc.vector.tensor_tensor(smax[:], smax[:], s_ps[:], op=ALU.max)
                mrow = ppool.tile([2, TQ], f32, tag="mrow", bufs=1)
                for tck in range(4):
                    tr = big_ps.tile([128, 128], f32, tag="big")
                    nc.tensor.transpose(tr[:], smax[:, tck*128:(tck+1)*128], ident[:])
                    mcol = ppool.tile([128, 1], f32, tag="mcol")
                    nc.vector.tensor_reduce(mcol[:], tr[:], axis=AX.X, op=ALU.max)
                    nc.sync.dma_start(mrow[0:1, tck*128:(tck+1)*128], mcol[:])
                    nc.sync.dma_start(mrow[1:2, tck*128:(tck+1)*128], mcol[:])
                mneg = ppool.tile([2, TQ], f32r, tag="mneg", bufs=1)
                nc.vector.tensor_scalar(out=mneg[:], in0=mrow[:], scalar1=-1.0,
                                        scalar2=None, op0=ALU.mult)
                # pass 2
                l_ps = att_ps.tile([2, TQ], f32, tag="l")
                o_ps0 = att_ps.tile([128, TQ], f32, tag="o0")
                o_ps1 = att_ps.tile([128, TQ], f32, tag="o1")
                for ut in range(NU):
                    s_ps = big_ps.tile([128, TQ], f32, tag="big")
                    nc.tensor.matmul(s_ps[:], onz[:], mneg[:], start=True, stop=False)
                    nc.tensor.matmul(s_ps[:], kslice(0, ut), qh0, start=False, stop=False)
                    nc.tensor.matmul(s_ps[:], kslice(1, ut), qh1, start=False, stop=True)
                    p = pp3.tile([128, TQ], f32r, tag="p")
                    nc.scalar.activation(p[:], s_ps[:], AF.Exp)
                    nc.tensor.matmul(l_ps[:], ones[:, 0:2], p[:],
                                     start=(ut == 0), stop=(ut == NU - 1))
                    nc.tensor.matmul(o_ps0[:], Vh[:, ut, 0:128], p[:],
                                     start=(ut == 0), stop=(ut == NU - 1))
                    nc.tensor.matmul(o_ps1[:], Vh[:, ut, 128:256], p[:],
                                     start=(ut == 0), stop=(ut == NU - 1))
                linv = ppool.tile([2, TQ], f32r, tag="linv")
                with nc.allow_low_precision(reason="f32r rounding feeds matmul"):
                    nc.vector.reciprocal(linv[:], l_ps[:])
                lbc_ps = big_ps.tile([128, TQ], f32, tag="big")
                nc.tensor.matmul(lbc_ps[:], onz[:], linv[:], start=True, stop=True)
                lbc = ppool.tile([128, TQ], f32, tag="lbcs", bufs=1)
                nc.vector.tensor_copy(lbc[:], lbc_ps[:])
                nc.vector.tensor_tensor(oT[:, 2*h, :], o_ps0[:], lbc[:], op=ALU.mult)
                nc.vector.tensor_tensor(oT[:, 2*h+1, :], o_ps1[:], lbc[:], op=ALU.mult)

        kvctx.close()
        # ---- o-proj + residual + sensor partial ----
        fsm_ps = big_ps.tile([128, TQ], f32, tag="big")
        nc.tensor.matmul(fsm_ps[:], onz[:], fsm[:], start=True, stop=True)
        fsm_bc = sb.tile([128, TQ], f32)
        nc.vector.tensor_copy(fsm_bc[:], fsm_ps[:])
        sens = sb.tile([128, KT], f32)
        for m in range(KT):
            ps = big_ps.tile([128, TQ], f32, tag="big")
            for kt in range(KT):
                wt = wpool.tile([128, 128], f32r, tag="wt")
                nc.sync.dma_start(wt[:], t["Wo_at"][kt*128:(kt+1)*128, m*128:(m+1)*128])
                nc.tensor.matmul(ps[:], wt[:], oT[:, kt, :],
                                 start=(kt == 0), stop=(kt == KT - 1))
            mix = ppool.tile([128, TQ], f32, tag="mix")
            nc.vector.tensor_tensor(mix[:], ps[:], xT[:, m, :], op=ALU.add)
            mixf = ppool.tile([128, TQ], f32, tag="mixf")
            nc.vector.tensor_tensor(mixf[:], mix[:], fsm_bc[:], op=ALU.mult)
            nc.vector.tensor_reduce(sens[:, m:m+1], mixf[:], axis=AX.X, op=ALU.add)
        nc.sync.dma_start(
            t["senb"][:].rearrange("(m p) -> p m", p=128),
            sens[:])
        nc.gpsimd.collective_compute(
            "AllReduce", ALU.add, replica_groups=t["RG"],
            ins=[t["senb"][:].opt()], outs=[t["senar"][:].opt()])
        nc.sync.dma_start(t["sensor"][:], t["senar"][:])

        # ---- Xin rows 0,1 ----
        sens_row = sb.tile([1, D], f32)
        nc.sync.dma_start(sens_row[:], t["senar"][:].unsqueeze(0))
        eps_t = sb.tile([1, 1], f32)
        nc.vector.memset(eps_t[:], 1e-6)
        XinIO = sb.tile([128, KT, 2], f32)
        for r in range(2):
            y = rowp.tile([1, D], f32, tag="y")
            nc.vector.tensor_tensor(y[:], Mio[0:1, r*D:(r+1)*D], sens_row[:], op=ALU.add)
            ysq = rowp.tile([1, D], f32, tag="ysq")
            ssq = rowp.tile([1, 1], f32, tag="ssq")
            nc.scalar.activation(ysq[:], y[:], AF.Square, accum_out=ssq[:])
            sd = rowp.tile([1, 1], f32, tag="sd")
            nc.scalar.activation(sd[:], ssq[:], AF.Sqrt, bias=eps_t[:], scale=1.0 / D)
            rinv = rowp.tile([1, 1], f32, tag="rinv")
            nc.vector.reciprocal(rinv[:], sd[:])
            yn = rowp.tile([1, D], f32, tag="yn")
            nc.vector.tensor_scalar(out=yn[:], in0=y[:], scalar1=rinv[:],
                                    scalar2=None, op0=ALU.mult)
            yg = rowp.tile([1, D], f32, tag="yg")
            nc.vector.tensor_tensor(yg[:], yn[:], gamma[:], op=ALU.mult)
            nc.sync.dma_start(t["xiob"][r:r+1, :], yg[:])
            nc.sync.dma_start(
                XinIO[:, :, r],
                t["xiob"][r:r+1, :].rearrange("one (kt p) -> (one p) kt", p=128))
        for a in range(2):
            rwkv_region(a, XinIO, a)

        nc.gpsimd.collective_compute(
            "AllReduce", ALU.add, replica_groups=t["RG"],
            ins=[t["hpb"][:].opt()], outs=[t["hpar"][:].opt()])
        nc.sync.dma_start(t["hsum"][:], t["hpar"][:])

        # ---- head: motor -> pi, facets, u ----
        wfpool = ctx.enter_context(tc.tile_pool(name="wf", bufs=1))
        WfC = wfpool.tile([128, KF, KT, 128], f32)
        nc.sync.dma_start(WfC[:], t["WfC"][:].transpose([2, 0, 1, 3]))
        hs1 = sb.tile([128, KT], f32)
        nc.sync.dma_start(
            hs1[:],
            t["hpar"][1:2, :].rearrange("one (kt p) -> (one p) kt", p=128))
        motor = sb.tile([128, KT], f32)
        nc.vector.tensor_tensor(motor[:], hs1[:], XinIO[:, :, 1], op=ALU.add)

        pi_ps = rwk_ps.tile([128, NVT], f32, tag="rwk")
        for kt in range(KT):
            nc.tensor.matmul(pi_ps[0:KF, 0:1], wpiT[:, kt, :], motor[:, kt:kt+1],
                             start=(kt == 0), stop=(kt == KT - 1))
        pi_col = ppool.tile([KF, 1], f32, tag="picol")
        nc.vector.tensor_copy(pi_col[:], pi_ps[0:KF, 0:1])
        prow = ppool.tile([1, KF], f32, tag="prow")
        nc.sync.dma_start(prow[:], pi_col[:])
        pmax = ppool.tile([1, 1], f32, tag="pmax")
        nc.vector.tensor_reduce(pmax[:], prow[:], axis=AX.X, op=ALU.max)
        pmaxn = ppool.tile([1, 1], f32, tag="pmaxn")
        nc.vector.tensor_scalar(out=pmaxn[:], in0=pmax[:], scalar1=-1.0,
                                scalar2=None, op0=ALU.mult)
        pie = ppool.tile([1, KF], f32, tag="pie")
        pis = ppool.tile([1, 1], f32, tag="pis")
        nc.scalar.activation(pie[:], prow[:], AF.Exp, bias=pmaxn[:], accum_out=pis[:])
        pinv = ppool.tile([1, 1], f32, tag="pinv")
        nc.vector.reciprocal(pinv[:], pis[:])
        pirow = ppool.tile([1, KF], f32, tag="pirow")
        nc.vector.tensor_scalar(out=pirow[:], in0=pie[:], scalar1=pinv[:],
                                scalar2=None, op0=ALU.mult)

        F_sb = sb.tile([128, KF], f32)
        for kf in range(KF):
            f_ps = rwk_ps.tile([128, NVT], f32, tag="rwk")
            for kt in range(KT):
                nc.tensor.matmul(f_ps[:, 0:1], WfC[:, kf, kt, :], motor[:, kt:kt+1],
                                 start=(kt == 0), stop=(kt == KT - 1))
            nc.scalar.activation(F_sb[:, kf:kf+1], f_ps[:, 0:1], AF.Tanh)
        nc.sync.dma_start(t["pib"][:], pirow[:])
        pib_sb = sb.tile([128, KF], f32)
        nc.sync.dma_start(pib_sb[:], t["pib"][:].unsqueeze(0).to_broadcast([128, KF]))
        uprod = ppool.tile([128, KF], f32, tag="uprod")
        nc.vector.tensor_tensor(uprod[:], F_sb[:], pib_sb[:], op=ALU.mult)
        u_sb = ppool.tile([128, 1], f32, tag="usb")
        nc.vector.tensor_reduce(u_sb[:], uprod[:], axis=AX.X, op=ALU.add)
        nc.sync.dma_start(t["ub"][:], u_sb[:])
        nc.sync.dma_start(t["dbg_u"][:], u_sb[:])
        nc.sync.dma_start(t["dbg_pi"][:], pirow[:])
        nc.sync.dma_start(t["dbg_F"][:], F_sb[:])
        nc.sync.dma_start(
            t["dbg_motor"][:].rearrange("(kt p) -> p kt", p=128), motor[:])
        nc.gpsimd.collective_compute(
            "AllGather", ALU.bypass, replica_groups=t["RG"],
            ins=[t["ub"][:].opt()], outs=[t["uag"][:].opt()])
        uT = sb.tile([128, KT], f32)
        nc.sync.dma_start(
            uT[:], t["uag"][:].rearrange("(kt p) -> p kt", p=128))

        # ---- logits ----
        Lg = sb.tile([128, NVT], f32)
        lg_ps = rwk_ps.tile([128, NVT], f32, tag="rwk")
        with tc.tile_pool(name="wout", bufs=80) as wopool:
            for vt in range(NVT):
                for kt in range(KT):
                    wt_o = wopool.tile([128, 128], f32, tag="stripe")
                    nc.sync.dma_start(
                        wt_o[:], t["WoutC"][kt*128:(kt+1)*128,
                                            vt*128:(vt+1)*128])
                    nc.tensor.matmul(lg_ps[:, vt:vt+1], wt_o[:],
                                     uT[:, kt:kt+1],
                                     start=(kt == 0), stop=(kt == KT - 1))
        nc.vector.tensor_copy(Lg[:], lg_ps[:])
        nc.sync.dma_start(
            t["logits"][:].rearrange("(vt p) -> p vt", p=128),
            Lg[:])


def _host_prep(inputs):
    f = np.float32
    tokens = np.asarray(inputs["tokens"])
    focus = np.asarray(inputs["focus_map"], f)
    H_prev = np.asarray(inputs["H_prev"], f)
    pos = np.stack([np.arange(T, dtype=f), np.arange(T, dtype=f) / (T + 1e-9)], -1)
    x = np.asarray(inputs["embed_W"], f)[tokens] + pos @ np.asarray(inputs["W_pos"], f)
    fsm = np.exp(focus - focus.max()); fsm = (fsm / fsm.sum()).astype(f)

    sc = H_prev @ np.asarray(inputs["w_gate"], f)
    top = np.argsort(-sc, kind="stable")[:KACT]
    reg_mask = np.zeros(R, bool); reg_mask[top] = True; reg_mask[[IO_S, IO_M]] = True
    active = np.where(reg_mask)[0]
    A = len(active)

    Hm = np.where(np.asarray(inputs["reg_mask_prev"])[:, None], H_prev, 0.0).astype(f)
    nbr = np.asarray(inputs["nbr_idx"])
    rc = np.asarray(inputs["reg_coords"], f)
    dist = np.linalg.norm(rc[:, None, :] - rc[nbr], axis=-1)
    w = np.exp(-dist - (-dist).max(1, keepdims=True)); w = (w / w.sum(1, keepdims=True)).astype(f)
    Magg = np.einsum('rn,rnd->rd', w, Hm[nbr])
    M_act = (Magg[active] @ np.asarray(inputs["W_route"], f))
    gamma = np.asarray(inputs["gamma"], f)

    XinN = np.zeros((A, D), f)
    for a, r in enumerate(active):
        if r not in (IO_S, IO_M):
            v = M_act[a]
            XinN[a] = v * gamma * (1.0 / np.sqrt(np.mean(v * v) + 1e-6))
    spos = f(float(inputs["step_k"]) / float(max(1, KINNER - 1)))
    return dict(x=x.astype(f), fsm=fsm, reg_mask=reg_mask, active=active, A=A,
                M_act=M_act, XinN=XinN, spos=spos, gamma=gamma)


def kernel(**inputs):
    f = np.float32
    hp = _host_prep(inputs)
    A, active = hp["A"], hp["active"]
    x, fsm = hp["x"], hp["fsm"]

    Wr = np.asarray(inputs["Wr"], f); Wv = np.asarray(inputs["Wv"], f)
    Wo = np.asarray(inputs["Wo"], f); b_step = np.asarray(inputs["b_step"], f)
    Wf = np.asarray(inputs["Wf"], f); w_pi = np.asarray(inputs["w_pi"], f)
    W_out = np.asarray(inputs["W_out"], f)
    Wout_pad = np.zeros((D, VPAD), f); Wout_pad[:, :V] = W_out

    XinNT = np.ascontiguousarray(
        hp["XinN"].T.reshape(KT, 128, A).transpose(1, 0, 2))

    common = {
        "Wq": np.asarray(inputs["Wq"], f) / np.sqrt(float(HD)),
        "Wk": np.asarray(inputs["Wk"], f),
        "Wv_at": np.asarray(inputs["Wv_attn"], f),
        "Wo_at": np.asarray(inputs["Wo_attn"], f),
        "ones": np.ones((128, 128), f), "ident": np.eye(128, dtype=f),
        "onz": np.concatenate([np.ones((1, 128), f), np.zeros((1, 128), f)]),
        "gammaR": hp["gamma"].reshape(1, D),
        "MioR": np.ascontiguousarray(hp["M_act"][0:2]),
        "XinNT": XinNT,
        "wpiT": np.ascontiguousarray(w_pi.T),
    }
    in_maps = []
    for c in range(NCORES):
        esl = slice(c * EC, (c + 1) * EC)
        m = dict(common)
        m["xT"] = np.ascontiguousarray(x[c*TQ:(c+1)*TQ].T)
        m["fsm"] = np.concatenate([fsm[c*TQ:(c+1)*TQ].reshape(1, TQ),
                                   np.zeros((1, TQ), f)])
        m["WrC"] = np.ascontiguousarray(
            Wr[active][:, :, esl].reshape(A, KT, 128, EC))
        m["WvC"] = np.ascontiguousarray(
            Wv[active][:, :, esl].reshape(A, KT, 128, EC))
        m["WoC"] = np.ascontiguousarray(
            Wo[active][:, esl, :].transpose(0, 2, 1).reshape(A, KT, 128, EC)
            .transpose(0, 1, 3, 2))
        m["bstepC"] = np.ascontiguousarray((hp["spos"] * b_step[active][:, esl]).T)
        m["WfC"] = np.ascontiguousarray(Wf[:, :, esl].reshape(KF, KT, 128, EC))
        m["WoutC"] = np.ascontiguousarray(Wout_pad[:, c*VC:(c+1)*VC])
        in_maps.append(m)

    if A not in _prog_cache:
        _prog_cache[A] = _build(A)
    nc = _prog_cache[A]
    res = run_bass_kernel_spmd(nc, in_maps, list(range(NCORES)))
    global LAST_RES
    LAST_RES = res

    logits_pad = np.concatenate([res.results[c]["logits"] for c in range(NCORES)])
    logits = logits_pad[:V]
    sensor = res.results[0]["sensor"]
    hsum = res.results[0]["hsum"]

    gamma = hp["gamma"]
    Xin = np.zeros((A, D), f)
    for a, r in enumerate(active):
        v = hp["M_act"][a] + (sensor if r in (IO_S, IO_M) else 0.0)
        Xin[a] = v * gamma * (1.0 / np.sqrt(np.mean(v * v) + 1e-6))
    H_act = hsum + Xin
    H_cur = np.zeros((R, D), f); H_cur[active] = H_act
    reg_mask = hp["reg_mask"]
    motor = H_act[1]
    ws = H_act.sum(0) / max(int(reg_mask.sum()), 1)
    rtd = np.float32(motor @ np.asarray(inputs["w_rtd"], f))
    return (H_cur.astype(f), reg_mask, logits.astype(f), rtd,
            ws.astype(f), motor.astype(f))
